# revision 70
# baseline (speedup 1.0000x reference)
"""Trainium2 Bass kernel for nn_FNORCF1d (FNO + Chebyshev feature transform, 1D).

Sharding: data-parallel over batch — core b computes batch element b (B=8,
8 cores). The only cross-core data is the CFT per-segment min/max: one tiny
8-byte AllGather per segment, launched from the previous layer's tail so its
latency hides under the gate/update phase.

Math restructurings:
  - rfft keeps only 32 modes -> forward DFT accumulates hT-chunk-stationary
    matmuls against a [128, 64] cos|-sin moving block, yielding hf in [c, k]
    layout directly (no post-transpose).
  - irfft of a 32-mode spectrum -> matmul with [64, L] (2/L-scaled cos/-sin).
  - cheb: filt.mean(-1) collapses to per-(c, s) coefficients gamma_p over
    the 8 power sums S_p = sum_seg xn^p (x*T_m expanded in the power basis
    on the host).  Per segment the Act engine materializes xn/xn^2/xn^4 with
    fused accumulation (S1/S2/S4/S6/S8 via Identity/Square passes), DVE
    materializes xn^3/xn^5/xn^7 and reduces S3/S5/S7.
  - gate matmul's x_cft half is piecewise-constant per segment -> folded into
    the sigmoid bias.
Precision: fp32 h carry and normalization extrema (reference-exact min/max);
bf16 weights/moving operands for all per-layer matmuls (conv/gate/fc1/fc2
included) with fp32 PSUM accumulation; rel err vs reference ~8e-3 against a
2e-2 gate.
Engines: Act carries gelu/sigmoid/CFT-squares (the layer-critical chain),
DVE the products/reduces/updates, Pool the cross-lane maxes + collectives,
PE everything matmul-shaped.  Layer boundaries are software-pipelined:
h16 copies, per-segment extrema, and the AllGathers all ride the update
loop tail; segs 1-3 collectives launch at the next layer's front.
"""

import functools
import os
from contextlib import ExitStack

import numpy as np
from ml_dtypes import bfloat16

import concourse.bass as bass
import concourse.bacc as bacc
import concourse.mybir as mybir
import concourse.tile as tile
from concourse.bass_utils import run_bass_kernel_spmd

F32 = mybir.dt.float32
BF16 = mybir.dt.bfloat16
AF = mybir.ActivationFunctionType
ALU = mybir.AluOpType
AX = mybir.AxisListType

B, L, W, MODES, NL, SEG, CM, H1 = 8, 8192, 128, 32, 4, 4, 8, 128
LS = L // SEG           # 2048
NJ = L // 512           # 16 chunks of 512
NC128 = L // 128        # 64 chunks of 128
K2 = 2 * MODES          # 64 interleaved (re, im) rows


def _host_consts():
    """Input-independent constants (DFT matrices, identity, ones)."""
    lg = np.arange(L)
    kg = np.arange(MODES)
    ang = 2.0 * np.pi * np.outer(lg, kg) / L          # [L, 32]
    # forward: hf[c, k'] = sum_l hT[l, c] F[l, k']; F = (cos | -sin) [L, 64].
    # Chunked moving layout: fcat[:, c*64:(c+1)*64] = F[c*128:(c+1)*128, :].
    fall = np.empty((L, K2), np.float32)
    fall[:, :MODES] = np.cos(ang)
    fall[:, MODES:K2] = -np.sin(ang)
    fcat = fall.reshape(NC128, 128, K2).transpose(1, 0, 2).reshape(128, NC128 * K2)
    # inverse: sp[l] = sum_k coef/L * (re_k cos - im_k sin); rows interleaved,
    # padded to 128 contraction rows (rows 64.. are zero)
    coefk = (np.where(kg == 0, 1.0, 2.0) / L).astype(np.float32)
    minv = np.zeros((128, L), np.float32)
    minv[0:K2:2] = coefk[:, None] * np.cos(ang.T)
    minv[1:K2:2] = -coefk[:, None] * np.sin(ang.T)
    return {
        "fcat": fcat.astype(bfloat16),
        "minv": minv.astype(bfloat16),
        "ident_f": np.eye(128, dtype=np.float32),
        "ident_b": np.eye(128).astype(bfloat16),
        "ones128": np.ones((128, 128), np.float32),
    }


def _host_weights(inputs):
    """Reformat the model weights for the kernel."""
    spec = np.empty((NL, 128, MODES * 256), np.float32)
    for i in range(NL):
        for k in range(MODES):
            spec[i, :, k * 256: k * 256 + 128] = inputs["spec_wr"][i][:, :, k]
            spec[i, :, k * 256 + 128: (k + 1) * 256] = inputs["spec_wi"][i][:, :, k]

    # gamma: arg[c,s] = sum_p gamma_p[c,s] * S_p[c,s], where S_p = sum over
    # the segment of xn^p (xn = a_s h + b_s), p = 1..8.  filt.mean(-1) =
    # sum_m cb[s,m,c] * mean(xn*T_m(xn)); x*T_m in the power basis is Ct.
    cb = inputs["cheb_w"].mean(-1)                    # [NL, SEG, CM, W]
    Ct = np.zeros((CM, 9))
    Ct[0, 1] = 1
    Ct[1, 2] = 1
    Ct[2, 3], Ct[2, 1] = 2, -1
    Ct[3, 4], Ct[3, 2] = 4, -3
    Ct[4, 5], Ct[4, 3], Ct[4, 1] = 8, -8, 1
    Ct[5, 6], Ct[5, 4], Ct[5, 2] = 16, -20, 5
    Ct[6, 7], Ct[6, 5], Ct[6, 3], Ct[6, 1] = 32, -48, 18, -1
    Ct[7, 8], Ct[7, 6], Ct[7, 4], Ct[7, 2] = 64, -112, 56, -7
    gamma = np.einsum('lsmc,mp->lspc', cb, Ct[:, 1:9]) / LS  # [NL, SEG, 8, W]
    # permute p to the device accS column order [S1,S2,S6,S4,S3,S5,S7,S8]
    gamma = gamma[:, :, [0, 1, 5, 3, 2, 4, 6, 7], :]
    gamma_d = gamma.transpose(3, 0, 1, 2).reshape(W, NL * SEG * 8)

    fc0w = np.zeros((128, 128), np.float32)
    fc0w[0:2] = inputs["fc0_w"]
    fc2w = np.zeros((128, 128), np.float32)
    fc2w[:, 0:1] = inputs["fc2_w"]

    return {
        "spec": spec.astype(bfloat16),                               # [NL,128,8192]
        "convw": np.concatenate(list(inputs["conv_w"]), axis=1).astype(bfloat16),
        "convb": inputs["conv_b"].T.astype(np.float32),              # [128, NL]
        "gwa": np.concatenate([inputs["gate_w"][i][:W] for i in range(NL)], axis=1).astype(bfloat16),
        "gwb": np.concatenate([inputs["gate_w"][i][W:] for i in range(NL)], axis=1).astype(np.float32),
        "gateb": inputs["gate_b"].T.astype(np.float32),              # [128, NL]
        "gamma": gamma_d.astype(np.float32),
        "fc0w": fc0w,
        "fc0b": inputs["fc0_b"].reshape(W, 1).astype(np.float32),
        "fc1w": inputs["fc1_w"].astype(bfloat16),                    # [128, 128]
        "fc1b": inputs["fc1_b"].reshape(H1, 1).astype(np.float32),
        "fc2w": fc2w.astype(bfloat16),
        "fc2b": inputs["fc2_b"].reshape(1, 1).astype(np.float32),
    }


# Constants are packed host-side into two blobs (one DMA each).
_B16_ORDER = [("fcat", NC128 * K2), ("minv", L), ("ident_b", 128),
              ("convw", NL * 128), ("gwa", NL * 128), ("fc1w", 128),
              ("fc2w", 128)]
_B32_ORDER = [("ident_f", 128), ("ones128", 128), ("gwb", NL * 128),
              ("convb", NL), ("gateb", NL), ("gamma", NL * 32),
              ("fc0w", 128), ("fc0b", 1), ("fc1b", 1), ("fc2b", 1)]
_NB16 = sum(n for _, n in _B16_ORDER)
_NB32 = sum(n for _, n in _B32_ORDER)

_SPECS = {
    # name: (shape, dtype)
    "xt": ((2, L), F32),
    "cb16": ((128, _NB16), BF16),
    "cb32": ((128, _NB32), F32),
    "spec": ((NL, 128, MODES * 256), BF16),
}


def _host_all(inputs):
    """Packed input map (minus xt): cb16/cb32 blobs + spec."""
    pieces = dict(_host_consts())
    pieces.update(_host_weights(inputs))
    b16 = np.concatenate(
        [np.asarray(pieces[nm], dtype=bfloat16).reshape(128, n)
         for nm, n in _B16_ORDER], axis=1)
    b32 = []
    for nm, n in _B32_ORDER:
        a = np.asarray(pieces[nm], dtype=np.float32)
        if a.shape[0] != 128:                       # fc2b is [1, 1]
            a = np.broadcast_to(a, (128, n)).copy()
        b32.append(a.reshape(128, n))
    return {"cb16": np.ascontiguousarray(b16),
            "cb32": np.ascontiguousarray(np.concatenate(b32, axis=1)),
            "spec": np.ascontiguousarray(pieces["spec"])}


def _emit(tc, ap, ctx):
    """Emit the whole per-core program inside TileContext tc."""
    lvl = int(os.environ.get("KBISECT", "99"))
    alt = int(os.environ.get("KCFTALT", "3"))
    nc = tc.nc
    ec = ctx.enter_context

    cpool = ec(tc.tile_pool(name="const", bufs=1))
    spool = ec(tc.tile_pool(name="spec", bufs=1))
    state = ec(tc.tile_pool(name="state", bufs=1))
    cft = ec(tc.tile_pool(name="cft", bufs=1))
    small = ec(tc.tile_pool(name="small", bufs=2))
    gpool = ec(tc.tile_pool(name="gate", bufs=4))
    dpool = ec(tc.tile_pool(name="dram", bufs=4, space="DRAM"))
    ptr = ec(tc.tile_pool(name="ptr", bufs=2, space="PSUM"))
    psmall = ec(tc.tile_pool(name="psmall", bufs=2, space="PSUM"))
    pbig = ec(tc.tile_pool(name="pbig", bufs=2, space="PSUM"))
    pmisc = ec(tc.tile_pool(name="pmisc", bufs=1, space="PSUM"))

    # ---- load constants (two blob DMAs; small f32 blob first so fc0's
    # weights arrive before the big bf16 blob transfer) ----
    cb32_t = cpool.tile([128, _NB32], F32, tag="cb32", name="cb32")
    nc.sync.dma_start(cb32_t[:], ap["cb32"][:])
    xcs = []
    for g in range(4):
        xc = small.tile([2, 2048], F32, tag="xc", bufs=2)
        nc.sync.dma_start(xc[:], ap["xt"][:, g * 2048:(g + 1) * 2048])
        xcs.append(xc)
    cb16_t = cpool.tile([128, _NB16], BF16, tag="cb16", name="cb16")
    nc.sync.dma_start(cb16_t[:], ap["cb16"][:])
    tiles = {}
    off = 0
    for nm, n in _B16_ORDER:
        tiles[nm] = cb16_t[:, off:off + n]
        off += n
    off = 0
    for nm, n in _B32_ORDER:
        tiles[nm] = cb32_t[:, off:off + n]
        off += n
    fcat, minv = tiles["fcat"], tiles["minv"]
    ident_f, ident_b = tiles["ident_f"], tiles["ident_b"]
    ones128 = tiles["ones128"]
    convw, convb = tiles["convw"], tiles["convb"]
    gwa, gwb, gateb = tiles["gwa"], tiles["gwb"], tiles["gateb"]
    gamma = tiles["gamma"]
    fc0w, fc0b = tiles["fc0w"], tiles["fc0b"]
    fc1w, fc1b = tiles["fc1w"], tiles["fc1b"]
    fc2w, fc2b = tiles["fc2w"], tiles["fc2b"]

    h = state.tile([128, L], F32, tag="h")
    xfno = state.tile([128, L], BF16, tag="xfno")
    h16 = state.tile([128, L], BF16, tag="h16")
    hT = state.tile([128, L], BF16, tag="hT")
    ab2z = state.tile([128, 8], F32, tag="ab2z")
    nc.gpsimd.memset(ab2z[:], 0.0)

    spec_t = [None] * NL
    spec_t[0] = spool.tile([128, MODES * 256], BF16, tag="spec", name="spec0")
    nc.sync.dma_start(spec_t[0][:], ap["spec"][0])

    # Per-segment boundary: once h[:, seg] is final, copy it to h16 and
    # launch a tiny per-segment SBUF AllGather of [min_s, -max_s].  The
    # per-segment split pipelines the collective latency into the previous
    # layer's tail (segment s is needed only at CFT step s of the next
    # layer).
    def emit_seg_boundary(s, pairs, with_cc=True):
        sl = slice(s * LS, (s + 1) * LS)
        nc.vector.tensor_copy(h16[:, sl], h[:, sl])
        if with_cc:
            emit_cc_only(s, pairs)

    def emit_cc_only(s, pairs):
        sl = slice(s * LS, (s + 1) * LS)
        colm = small.tile([128, 1], F32, tag="colm")
        nc.vector.tensor_reduce(colm[:],
                                h[:, sl].rearrange("p (o q) -> p o q", o=1),
                                AX.X, ALU.min)
        pc = pmisc.tile([1, 128], F32, tag="misc", bufs=1)
        nc.tensor.transpose(pc[:], colm[:], ident_f[:])
        mb2 = small.tile([1, 2], F32, tag="mb2")
        nc.vector.tensor_reduce(mb2[0:1, 0:1], pc[:], AX.X, ALU.min)
        nc.gpsimd.tensor_reduce(mb2[0:1, 1:2], h[:, sl], AX.XYZWC, ALU.max)
        nc.vector.tensor_scalar(mb2[0:1, 1:2], mb2[0:1, 1:2],
                                -1.0, None, ALU.mult)
        cci = dpool.tile([2, 1], F32, tag="cci", bufs=4)
        cco = dpool.tile([2 * B, 1], F32, tag="cco", bufs=4)
        nc.gpsimd.dma_start(cci[:].rearrange("e one -> one e"), mb2[:])
        nc.gpsimd.collective_compute(
            "AllGather", ALU.bypass,
            ins=[cci[:].opt()], outs=[cco[:].opt()],
            replica_groups=[list(range(B))],
        )
        pairs.append(cco)

    def emit_absb(s, cco, absb):
        """Fold the gathered [min, -max] pairs and broadcast a_s/b_s."""
        g16 = small.tile([1, 2 * B], F32, tag="g16")
        nc.sync.dma_start(
            g16[:], cco[:].rearrange("(core pair) one -> one (core pair)", pair=2))
        g2 = small.tile([1, 2], F32, tag="g2")
        gv = g16[:].rearrange("one (core pair) -> one core pair", pair=2)
        nc.vector.tensor_reduce(
            g2[:], gv.rearrange("one core pair -> one pair core"),
            AX.X, ALU.min)
        negd = small.tile([1, 1], F32, tag="negd")
        nc.vector.tensor_tensor(negd[:], g2[:, 0:1], g2[:, 1:2], ALU.add)
        inv = small.tile([1, 1], F32, tag="invd")
        nc.vector.reciprocal(inv[:], negd[:])          # -1/(mx-mn)
        nc.vector.tensor_scalar(ab2z[0:1, 2 * s:2 * s + 1], inv[:],
                                -2.0, None, ALU.mult)
        m1 = small.tile([1, 1], F32, tag="m1")
        nc.vector.tensor_tensor(m1[:], g2[:, 0:1], inv[:], ALU.mult)
        nc.vector.tensor_scalar(ab2z[0:1, 2 * s + 1:2 * s + 2], m1[:],
                                2.0, -1.0, ALU.mult, ALU.add)
        pab = pmisc.tile([128, 2], F32, tag="misc", bufs=1)
        nc.tensor.matmul(pab[:], ones128[:], ab2z[:, 2 * s:2 * s + 2],
                         start=True, stop=True)
        nc.vector.tensor_copy(absb[:, 2 * s:2 * s + 2], pab[:])

    # ---- fc0: h = fc0_w.T @ xt + b (K=2 fp32; xt streamed per chunk) ----
    pairs = []
    for g in range(4):
        xc = xcs[g]
        for jj in range(4):
            j = 4 * g + jj
            js = slice(j * 512, (j + 1) * 512)
            p = pbig.tile([128, 512], F32, tag="xf", bufs=2)
            nc.tensor.matmul(p[:], fc0w[0:2, :], xc[:, jj * 512:(jj + 1) * 512],
                             start=True, stop=True)
            nc.scalar.activation(h[:, js], p[:], AF.Identity, bias=fc0b[:])
        emit_seg_boundary(g, pairs, with_cc=(g == 0))

    # ---- layers ----
    nlayers = 0 if lvl < 2 else (1 if lvl < 7 else (2 if lvl < 8 else NL))
    for i in range(nlayers):
        # --- transpose h16 (prepared by the previous layer's tail) into hT,
        # emitted first so the DVE PSUM->SBUF copies unblock the FNO chain ---
        for g in range(NC128 // 4):
            p = ptr.tile([128, 512], BF16, tag="tr", bufs=2)
            for t in range(4):
                c = g * 4 + t
                nc.tensor.transpose(p[:, t * 128:(t + 1) * 128],
                                    h16[:, c * 128:(c + 1) * 128], ident_b[:])
            nc.vector.tensor_copy(hT[:, g * 512:(g + 1) * 512], p[:])

        # launch the remaining per-segment collectives for this layer's
        # normalization (seg 0 was launched in the previous tail)
        for s in range(1, SEG):
            emit_cc_only(s, pairs)
        absb = small.tile([128, 2 * SEG], F32, tag="absb")

        if lvl < 3:
            continue
        # --- forward DFT: hf[c, k'] directly (hT chunks stationary,
        # 64-col cos|-sin moving; accumulate over 64 l-chunks) ---
        phf = psmall.tile([128, K2], F32, tag="sm", bufs=1)
        for c in range(NC128):
            nc.tensor.matmul(phf[:], hT[:, c * 128:(c + 1) * 128],
                             fcat[:, c * K2:(c + 1) * K2],
                             start=(c == 0), stop=(c == NC128 - 1))
        # rhs1 = interleave(hr, hi); rhs2 = interleave(-hi, hr)
        rhs1 = small.tile([128, K2], BF16, tag="rhs1")
        rhs2 = small.tile([128, K2], BF16, tag="rhs2")
        r1v = rhs1[:].rearrange("p (k two) -> p k two", two=2)
        r2v = rhs2[:].rearrange("p (k two) -> p k two", two=2)
        hrv = phf[:, 0:MODES].rearrange("p k -> p k ()")
        hiv = phf[:, MODES:K2].rearrange("p k -> p k ()")
        nc.vector.tensor_copy(r1v[:, :, 0:1], hrv)
        nc.vector.tensor_copy(r1v[:, :, 1:2], hiv)
        nc.vector.tensor_scalar(r2v[:, :, 0:1], hiv, -1.0, None, ALU.mult)
        nc.vector.tensor_copy(r2v[:, :, 1:2], hrv)

        # --- mode mix: om[o, (re_k, im_k)] ---
        pom = psmall.tile([128, K2], F32, tag="sm", bufs=1)
        for k in range(MODES):
            nc.tensor.matmul(pom[:, 2 * k:2 * k + 2],
                             spec_t[i][:, k * 256:k * 256 + 128],
                             rhs1[:, 2 * k:2 * k + 2], start=True, stop=False)
            nc.tensor.matmul(pom[:, 2 * k:2 * k + 2],
                             spec_t[i][:, k * 256 + 128:(k + 1) * 256],
                             rhs2[:, 2 * k:2 * k + 2], start=False, stop=True)
        om_sb = small.tile([128, 128], BF16, tag="om_sb")
        nc.vector.tensor_copy(om_sb[:, 0:K2], pom[:])
        pomT = psmall.tile([128, 128], BF16, tag="sm", bufs=1)
        nc.tensor.transpose(pomT[:], om_sb[:], ident_b[:])
        omT_sb = small.tile([128, 128], BF16, tag="omT_sb")
        nc.vector.memset(omT_sb[K2:128, :], 0.0)
        nc.vector.tensor_copy(omT_sb[0:K2, :], pomT[0:K2, :])

        # prefetch next layer's spectral weights
        if i + 1 < NL:
            spec_t[i + 1] = spool.tile([128, MODES * 256], BF16, tag="spec",
                                       name=f"spec{i + 1}")
            nc.gpsimd.dma_start(spec_t[i + 1][:], ap["spec"][i + 1])

        if lvl < 4:
            continue
        # --- spectral + conv -> gelu -> x_fno (emitted before the CFT so
        # the Act engine runs the gelu-table block contiguously) ---
        for j in range(NJ):
            js = slice(j * 512, (j + 1) * 512)
            p = pbig.tile([128, 512], F32, tag="xf", bufs=2)
            nc.tensor.matmul(p[:], omT_sb[:], minv[:, js], start=True, stop=False)
            nc.tensor.matmul(p[:], convw[:, i * 128:(i + 1) * 128], h16[:, js],
                             start=False, stop=True)
            nc.scalar.activation(xfno[:, js], p[:], AF.Gelu, bias=convb[:, i:i + 1])

        if lvl < 5:
            continue
        # --- CFT via even/odd power sums: S_p = sum_seg xn^p, p=1..8;
        # arg = sum_p gamma_p S_p; rec = tanh(arg).  Act computes t1/e/e2/S8
        # (Identity+Square, sigmoid-table-safe), DVE the four cross TTRs. ---
        # accS col order per segment: [S1, S2, S6, S4, S3, S5, S7, S8]
        # (gamma is host-permuted to match).  Act carries S1/S2/S4/S6/S8
        # via activation accum; DVE materializes o3=xn^3, x5, x7 (plain
        # TT, bf16) and one packed reduce yields S3/S5/S7.
        accS = small.tile([128, SEG * 8], F32, tag="accS")
        rec = small.tile([128, SEG], F32, tag="rec")
        # Software-pipelined: the da/db (S6/S8) Act passes for segment s are
        # emitted after t1/e/e2 of segment s+1, so the Act queue never waits
        # on the DVE o3-chain.
        pend = []

        def emit_dadb(o3p, e2p, sbp):
            da = cft.tile([128, LS], BF16, tag="da", bufs=1)
            nc.scalar.activation(da[:], o3p, AF.Square, accum_out=sbp[:, 2:3])
            db = cft.tile([128, LS], BF16, tag="da", bufs=1)
            nc.scalar.activation(db[:], e2p[:], AF.Square, accum_out=sbp[:, 7:8])

        for s in range(SEG):
            emit_absb(s, pairs.pop(0), absb)
        for s in range(SEG):
            hseg = h[:, s * LS:(s + 1) * LS]
            a_ap = absb[:, 2 * s:2 * s + 1]
            b_ap = absb[:, 2 * s + 1:2 * s + 2]
            sb = accS[:, s * 8:s * 8 + 8]
            t1 = cft.tile([128, LS], BF16, tag="t1", bufs=1)
            nc.scalar.activation(t1[:], hseg, AF.Identity, bias=b_ap, scale=a_ap,
                                 accum_out=sb[:, 0:1])
            e = cft.tile([128, LS], BF16, tag="e", bufs=1)
            nc.scalar.activation(e[:], hseg, AF.Square, bias=b_ap, scale=a_ap,
                                 accum_out=sb[:, 1:2])
            e2 = cft.tile([128, LS], BF16, tag="e2", bufs=2)
            nc.scalar.activation(e2[:], e[:], AF.Square, accum_out=sb[:, 3:4])
            o3t = cft.tile([128, LS], BF16, tag="o3", bufs=2)
            o3 = o3t[:]
            nc.vector.tensor_tensor(o3, t1[:], e[:], ALU.mult)
            q = cft.tile([128, 2 * LS], BF16, tag="q", bufs=1)
            nc.vector.tensor_tensor(q[:, 0:LS], o3, e[:], ALU.mult)
            nc.vector.tensor_tensor(q[:, LS:2 * LS], o3, e2[:], ALU.mult)
            nc.vector.tensor_reduce(sb[:, 4:5], o3t[:], AX.X, ALU.add)
            nc.vector.tensor_reduce(
                sb[:, 5:7], q[:].rearrange("p (j q) -> p j q", j=2),
                AX.X, ALU.add)
            if pend:
                emit_dadb(*pend.pop())
            pend.append((o3, e2, sb))
        emit_dadb(*pend.pop())
        # rec = tanh(sum_p gamma_p * S_p); gamma cols remapped on host to the
        # device accS order [S1,S2,S3,S4,S5,S7?? -> see _host_weights perm].
        prod = small.tile([128, SEG * 8], F32, tag="prod")
        nc.vector.tensor_tensor(prod[:], gamma[:, i * 32:(i + 1) * 32], accS[:], ALU.mult)
        ard = small.tile([128, SEG], F32, tag="ard")
        nc.vector.tensor_reduce(ard[:], prod[:].rearrange("p (s j) -> p s j", j=8),
                                AX.X, ALU.add)
        nc.scalar.activation(rec[:], ard[:], AF.Tanh)

        if lvl < 6:
            continue
        # --- gate bias from CFT: gs = gwB.T @ rec (+gate_b) ---
        pgs = pmisc.tile([128, SEG], F32, tag="misc", bufs=1)
        nc.tensor.matmul(pgs[:], gwb[:, i * 128:(i + 1) * 128], rec[:],
                         start=True, stop=True)
        biasg = small.tile([128, SEG], F32, tag="biasg")
        nc.vector.tensor_scalar(biasg[:], pgs[:], gateb[:, i:i + 1], None, ALU.add)

        # --- gate matmul + sigmoid + h update; per segment, fold the
        # next layer's h16/min/max into the tail and launch its AllReduce ---
        last = i + 1 >= nlayers
        for j in range(NJ):
            js = slice(j * 512, (j + 1) * 512)
            s = j // (NJ // SEG)
            pg = pbig.tile([128, 512], F32, tag="gate", bufs=2)
            nc.tensor.matmul(pg[:], gwa[:, i * 128:(i + 1) * 128], xfno[:, js],
                             start=True, stop=True)
            gchunk = gpool.tile([128, 512], BF16, tag="g")
            nc.scalar.activation(gchunk[:], pg[:], AF.Sigmoid, bias=biasg[:, s:s + 1])
            nc.vector.scalar_tensor_tensor(h[:, js], gchunk[:], rec[:, s:s + 1],
                                           xfno[:, js], ALU.mult, ALU.add)
            if j % 4 == 3:
                emit_seg_boundary(j // 4, pairs,
                                  with_cc=(not last) and j // 4 == 0)

    if lvl < 9:
        return
    # ---- tail: fc1 -> gelu -> fc2 (h16 maintained by the last layer) ----
    z = state.tile([128, L], BF16, tag="xfno")
    for q in range(4):
        yq = small.tile([1, 2048], F32, tag="yq", bufs=2)
        for jj in range(4):
            j = 4 * q + jj
            js = slice(j * 512, (j + 1) * 512)
            p = pbig.tile([128, 512], F32, tag="xf", bufs=2)
            nc.tensor.matmul(p[:], fc1w[:], h16[:, js], start=True, stop=True)
            nc.scalar.activation(z[:, js], p[:], AF.Gelu, bias=fc1b[:])
            p2 = pmisc.tile([128, 512], F32, tag="misc", bufs=1)
            nc.tensor.matmul(p2[0:1, :], fc2w[:, 0:1], z[:, js], start=True, stop=True)
            nc.scalar.activation(yq[0:1, jj * 512:(jj + 1) * 512], p2[0:1, :],
                                 AF.Identity, bias=fc2b[0:1, :])
        nc.gpsimd.dma_start(ap["y"][:, q * 2048:(q + 1) * 2048], yq[:])


@functools.lru_cache(maxsize=1)
def _build():
    nc = bacc.Bacc("TRN2", target_bir_lowering=False, debug=False, num_devices=B)
    ap = {}
    for name, (shape, dt_) in _SPECS.items():
        ap[name] = nc.dram_tensor(name, list(shape), dt_, kind="ExternalInput").ap()
    ap["y"] = nc.dram_tensor("y", [1, L], F32, kind="ExternalOutput").ap()
    with tile.TileContext(nc) as tc:
        with ExitStack() as ctx:
            _emit(tc, ap, ctx)
    nc.compile()
    return nc


def kernel(**inputs):
    inputs = {k: np.asarray(v) for k, v in inputs.items()}
    nc = _build()

    shared = _host_all(inputs)
    in_maps = []
    for b in range(B):
        m = dict(shared)
        m["xt"] = np.ascontiguousarray(inputs["x"][b].T.astype(np.float32))
        in_maps.append(m)

    res = run_bass_kernel_spmd(nc, in_maps, list(range(B)))
    out = np.stack([res.results[b]["y"].reshape(L, 1) for b in range(B)])
    return out.astype(np.float32)


if __name__ == "__main__":
    import reference  # only when run manually inside /root/problem

    inputs = reference.setup_inputs()
    out = kernel(**{k: np.asarray(v) for k, v in inputs.items()})
    print(out.shape, np.abs(out).max())

